# revision 1
# baseline (speedup 1.0000x reference)
"""FFT depthwise conv == direct 7x7 circular depthwise conv, on 8 TRN2 cores.

out[b,i,j,c] = sum_{u,v} wf[c,u,v] * x[b,(i+u-3)%H,(j+v-3)%W,c],  wf = kernel[:, ::-1, ::-1]

Sharding: data-parallel over batch (1 image per core). Host pre-pads each
image circularly to (C, 230, 230) and ships it in bf16, so every on-device
tile load is a plain contiguous-row DMA (no wrap handling on device).

Per core: partitions = 64 channels x 2 row-halves; 3 channel groups x 4
row-strips of 28 rows, each processed as two 14-row sub-strips:
  TensorE : N_PE_TAPS taps as diagonal-weight bf16 matmuls, fp32-accumulated
            in PSUM per 2-row bank tile (8 rotating single-bank tiles; a
            sub-strip's 7 banks never reuse a slot mid-sub-strip, so PE
            never stalls on same-sub-strip merges)
  VectorE : remaining taps as bf16 2-op MACs (tensor_scalar product in 4x
            mode + tensor_tensor add in 2x mode), then merges each PSUM bank
            with the accumulator into a bf16 output tile (fused downcast)
  ScalarE : copies the 6 overlapping halo rows from the previous strip tile
            (saves ~18% of input DMA) and issues half the DMAs (2nd HWDGE)
Odd-column taps are 2-byte-misaligned in bf16 and would break the DVE 2x/4x
modes, so they always go to the PE side of the split. Input/output DMAs are
row-chunked across both HWDGE queues; the next group's first tile is
prefetched one chunk per strip to keep group transitions off the critical
path. Built as bacc.Bacc (finalize() spills excess per-instruction sync
waits onto EventSemaphore instructions; engine slots are tiny).
"""

import os
import sys

for _p in ("/opt/trn_rl_repo", "/root/.axon_site/_ro/trn_rl_repo"):
    if os.path.isdir(_p) and _p not in sys.path:
        sys.path.insert(0, _p)

import numpy as np

import concourse.bacc as bacc
import concourse.bass as bass
import concourse.mybir as mybir
from concourse.bass_utils import run_bass_kernel_spmd
from concourse.tile import TileContext

F32 = mybir.dt.float32
F32R = mybir.dt.float32r
BF16 = mybir.dt.bfloat16

B, H, W, C, K = 8, 224, 224, 192, 7
NCORES = 8
PAD = K // 2          # 3
PH, PW = H + 2 * PAD, W + 2 * PAD  # 230, 230 padded image dims
HALF = H // 2         # 112 output rows per s-half
TH = 28               # output rows per strip (per half)
NSTRIP = HALF // TH   # 7
CG = 64               # channels per group
NG = C // CG          # 3
TROWS = TH + 2 * PAD  # 22 input rows per strip
TCOLS = PW            # 230 input cols per strip

# --- engine tap split (tunable) -------------------------------------------
# Odd-v taps are 2-byte-misaligned in the bf16 tile, which would knock the
# DVE out of its 2x perf mode -- so they are listed first and always land on
# the PE side of the split.
ALL_TAPS = sorted(
    ((u, v) for u in range(K) for v in range(K)),
    key=lambda t: (t[1] % 2 == 0, t[0], t[1]),
)
N_PE_TAPS = 34        # taps done on TensorE via diagonal matmuls (>= 21)
SUB = 14              # sub-strip rows (= 7 PSUM banks)
PE_TAPS = ALL_TAPS[:N_PE_TAPS]
VEC_TAPS = ALL_TAPS[N_PE_TAPS:]
USE_F32R = False
USE_BF16 = True

# DMA row-chunking: each chunk is one dma_start on its own queue/engine
IN_ROW_CHUNKS = [(0, 9), (9, 9), (18, 8), (26, 8)]     # covers TROWS=34
IN_ROW_CHUNKS_TAIL = [(6, 10), (16, 9), (25, 9)]       # rows 6..34 (halo 0..6 copied on-chip)
OUT_ROW_CHUNKS = [(0, 7), (7, 7)]                      # covers SUB=14


def _tap_idx(u, v):
    return u * K + v


def _add_dep(from_inst, to_inst):
    """Ordering-only (no-semaphore) dependency between two instructions."""
    import bass_rust as _br

    fi = getattr(from_inst, "ins", from_inst)
    ti = getattr(to_inst, "ins", to_inst)
    _br.add_dep_helper(fi, ti, sync=False, reason="seed-after-merge ordering")


def build_nc():
    # Bacc (not plain Bass): its compile() runs generate_event_semaphores,
    # which spills excess per-instruction sync waits onto EventSemaphore
    # instructions -- engine instructions only have 1 inline wait slot.
    nc = bacc.Bacc()
    xdt = BF16 if USE_BF16 else (F32R if USE_F32R else F32)
    odt = BF16 if USE_BF16 else F32
    x_d = nc.declare_dram_parameter("x", [C, PH, PW], xdt, isOutput=False)
    wvec_d = nc.declare_dram_parameter("wvec", [128, NG * K * K], F32, isOutput=False)
    wdiag_d = nc.declare_dram_parameter(
        "wdiag", [128, NG, K * K, 128], xdt, isOutput=False
    )
    out_d = nc.declare_dram_parameter("out", [C, H, W], odt, isOutput=True)

    mult = mybir.AluOpType.mult
    add = mybir.AluOpType.add
    act_copy = mybir.ActivationFunctionType.Copy

    with TileContext(nc) as tc:
        with (
            tc.tile_pool(name="consts", bufs=1) as cpool,
            tc.tile_pool(name="wdg", bufs=3) as wpool,
            tc.tile_pool(name="xin", bufs=4) as xpool,
            tc.tile_pool(name="xpre", bufs=2) as prepool,
            tc.tile_pool(name="accdp", bufs=3) as adpool,
            tc.tile_pool(name="tmpp", bufs=2) as tppool,
            tc.tile_pool(name="outp", bufs=4) as opool,
            tc.tile_pool(name="psum", bufs=8, space="PSUM") as ppool,
        ):
            wvec_sb = cpool.tile([128, NG * K * K], F32)
            nc.sync.dma_start(out=wvec_sb[:], in_=wvec_d[:])

            prev_merge = [None]  # last DVE merge instruction of previous strip

            # preload ALL groups' diagonal weights up front so group
            # transitions never wait on a 1.6 MB weight DMA stuck behind
            # the queued input DMAs
            def issue_in_dma(dst_tile, g, t, chunks=IN_ROW_CHUNKS):
                xh = x_d.tensor if hasattr(x_d, "tensor") else x_d
                base = g * CG * PH * PW + t * TH * PW
                for ci, (ra, nr) in enumerate(chunks):
                    srcap = bass.AP(
                        xh,
                        base + ra * PW,
                        [[HALF * PW, 2], [PH * PW, CG], [PW, nr], [1, TCOLS]],
                    )
                    eng = nc.sync if ci % 2 == 0 else nc.scalar
                    eng.dma_start(out=dst_tile[:, ra:ra + nr, :], in_=srcap)

            # first input tile FIRST so DVE work starts immediately; weight
            # loads follow on both queues
            xt00 = xpool.tile([128, TROWS, TCOLS], xdt, name="xt0_0", tag="xt")
            issue_in_dma(xt00, 0, 0)
            wdgs = []
            for g in range(NG):
                wdg = wpool.tile([128, K * K, 128], xdt, name=f"wdg{g}", tag="wdg")
                hkk = (K * K) // 2
                nc.sync.dma_start(out=wdg[:, 0:hkk, :], in_=wdiag_d[:, g, 0:hkk, :])
                nc.scalar.dma_start(
                    out=wdg[:, hkk:, :], in_=wdiag_d[:, g, hkk:, :]
                )
                wdgs.append(wdg)

            pre_tiles = {}
            for g in range(NG):
                wdg = wdgs[g]
                # prefetch the NEXT group's first input tile, one chunk per
                # strip of this group, so the transition tile is ready early
                # without ever bursting the DMA queues
                if g + 1 < NG:
                    pre = prepool.tile(
                        [128, TROWS, TCOLS], xdt, name=f"xpre{g + 1}", tag="xpre"
                    )
                    pre_tiles[g + 1] = pre

                for t in range(NSTRIP):
                    if g + 1 < NG:
                        # one staggered prefetch chunk for (g+1, t=0)
                        ci = t
                        ra, nr = IN_ROW_CHUNKS[ci]
                        xh = x_d.tensor if hasattr(x_d, "tensor") else x_d
                        base = (g + 1) * CG * PH * PW
                        srcap = bass.AP(
                            xh,
                            base + ra * PW,
                            [[HALF * PW, 2], [PH * PW, CG], [PW, nr], [1, TCOLS]],
                        )
                        eng = nc.sync if ci % 2 == 0 else nc.scalar
                        eng.dma_start(
                            out=pre_tiles[g + 1][:, ra:ra + nr, :], in_=srcap
                        )
                    if g == 0 and t == 0:
                        xt = xt00
                    elif t == 0 and g in pre_tiles:
                        xt = pre_tiles.pop(g)
                    else:
                        xt = xpool.tile(
                            [128, TROWS, TCOLS], xdt, name=f"xt{g}_{t}", tag="xt"
                        )
                        # rows 0..5 overlap the previous strip's tail: copy
                        # them on-chip (idle ScalarE) instead of re-DMAing
                        issue_in_dma(xt, g, t, chunks=IN_ROW_CHUNKS_TAIL)
                        nc.scalar.copy(
                            out=xt[:, 0:2 * PAD, :],
                            in_=prev_xt[:, TH:TH + 2 * PAD, :],
                        )
                    prev_xt = xt

                    # ---- two 14-row sub-strips per DMA strip: each uses
                    # exactly 7 PSUM banks (the full rotation), so PE never
                    # stalls waiting for same-strip merges
                    for sub in range(TH // SUB):
                        sb = sub * SUB
                        acc = adpool.tile(
                            [128, SUB, W], BF16, name=f"acc{g}_{t}_{sub}", tag="acc"
                        )
                        outt = opool.tile(
                            [128, SUB, W], odt, name=f"outt{g}_{t}_{sub}", tag="outt"
                        )
                        tmps = [
                            tppool.tile(
                                [128, SUB, W], BF16,
                                name=f"tmp{g}_{t}_{sub}_{j}", tag=f"tmp{j}",
                            )
                            for j in range(2)
                        ]

                        # ---- vector taps on DVE: all-bf16 2-op MACs.
                        # tensor_scalar products run in 4x mode, tensor_tensor
                        # adds in 2x mode -- beats the 1x-capped fused STT.
                        u0, v0 = VEC_TAPS[0]
                        ti0 = g * K * K + _tap_idx(u0, v0)
                        wv0 = wvec_sb[:, ti0:ti0 + 1]
                        seed = nc.vector.tensor_scalar(
                            acc[:],
                            xt[:, u0 + sb:u0 + sb + SUB, v0:v0 + W],
                            wv0,
                            None,
                            mult,
                        )
                        if prev_merge[0] is not None:
                            _add_dep(seed, prev_merge[0])
                        for j, (u, v) in enumerate(VEC_TAPS[1:]):
                            ti = g * K * K + _tap_idx(u, v)
                            wv = wvec_sb[:, ti:ti + 1]
                            tmp = tmps[j % 2]
                            nc.vector.tensor_scalar(
                                tmp[:],
                                xt[:, u + sb:u + sb + SUB, v:v + W],
                                wv,
                                None,
                                mult,
                            )
                            nc.vector.tensor_tensor(acc[:], acc[:], tmp[:], add)

                        # ---- TensorE taps: SUB/2 bank-tiles of 2 rows ----
                        n_pe = len(PE_TAPS)
                        for b8 in range(SUB // 2):
                            ps = ppool.tile(
                                [128, 512], F32, name=f"ps{g}_{t}_{sub}_{b8}", tag="ps"
                            )
                            row0 = 2 * b8
                            for ti, (u, v) in enumerate(PE_TAPS):
                                rhs = xt[:, u + sb + row0:u + sb + row0 + 2, v:v + W]
                                nc.tensor.matmul(
                                    ps[:, 0:2 * W],
                                    wdg[:, _tap_idx(u, v), :],
                                    rhs,
                                    start=(ti == 0),
                                    stop=(ti == n_pe - 1),
                                )
                            # merge psum + acc -> bf16 output tile (DVE)
                            ps3 = ps[:, 0:2 * W].rearrange("p (r w) -> p r w", r=2)
                            mg = nc.vector.scalar_tensor_tensor(
                                outt[:, row0:row0 + 2, :],
                                ps3,
                                1.0,
                                acc[:, row0:row0 + 2, :],
                                mult,
                                add,
                            )
                            if b8 == 0:
                                # the ordering hint for the next seed points at
                                # the FIRST merge: enough to cover transitive
                                # PE ticks, without serializing the next
                                # sub-strip behind PE's last bank
                                prev_merge[0] = mg

                        # ---- output DMA per sub-strip, row-chunked ----
                        oh = out_d.tensor if hasattr(out_d, "tensor") else out_d
                        obase = g * CG * H * W + (t * TH + sb) * W
                        for ci, (ra, nr) in enumerate(OUT_ROW_CHUNKS):
                            dst = bass.AP(
                                oh,
                                obase + ra * W,
                                [[HALF * W, 2], [H * W, CG], [W, nr], [1, W]],
                            )
                            eng = nc.scalar if ci % 2 == 0 else nc.sync
                            eng.dma_start(out=dst, in_=outt[:, ra:ra + nr, :])
    return nc


def _host_weights(kernel):
    """kernel: (C, K, K) -> (wvec [128, NG*49], wdiag [128, NG, 49, 128])."""
    wf = kernel[:, ::-1, ::-1].astype(np.float32)  # flipped: cross-correlation form
    cl = np.arange(128) % CG  # channel-local index per partition
    wvec = np.empty((128, NG * K * K), dtype=np.float32)
    wdiag = np.zeros((128, NG, K * K, 128), dtype=np.float32)
    eye = np.arange(128)
    for g in range(NG):
        wg = wf[g * CG:(g + 1) * CG].reshape(CG, K * K)  # (64, 49)
        wvec[:, g * K * K:(g + 1) * K * K] = wg[cl]
        wdiag[eye, g, :, eye] = wg[cl]
    return wvec, wdiag


_NC_CACHE = {}


def _get_nc():
    if "nc" not in _NC_CACHE:
        nc = build_nc()
        # Bacc passes (register alloc, EventSemaphore wait-splitting, ...)
        # run in finalize(); the pjrt path serializes the module as-is, so
        # finalize here before handing it off.
        nc.finalize()
        _NC_CACHE["nc"] = nc
    return _NC_CACHE["nc"]


def run(x, kernel, trace=False, **kw):
    assert x.shape == (B, H, W, C) and kernel.shape == (C, K, K)
    nc = _get_nc()
    xT = np.ascontiguousarray(x.transpose(0, 3, 1, 2)).astype(np.float32)  # (B,C,H,W)
    xTp = np.pad(xT, ((0, 0), (0, 0), (PAD, PAD), (PAD, PAD)), mode="wrap")
    xTp = np.ascontiguousarray(xTp)
    wvec, wdiag = _host_weights(np.asarray(kernel))
    if USE_BF16:
        import ml_dtypes

        xTp = xTp.astype(ml_dtypes.bfloat16)
        wdiag = wdiag.astype(ml_dtypes.bfloat16)
    in_maps = [{"x": xTp[b], "wvec": wvec, "wdiag": wdiag} for b in range(NCORES)]
    res = run_bass_kernel_spmd(nc, in_maps, list(range(NCORES)), trace=trace, **kw)
    out = np.stack(
        [np.asarray(res.results[b]["out"]).astype(np.float32) for b in range(NCORES)]
    )
    out = np.ascontiguousarray(out.transpose(0, 2, 3, 1)).astype(np.float32)
    return out, res


def kernel(x, kernel):
    out, _ = run(np.asarray(x), np.asarray(kernel))
    return out



# revision 2
# speedup vs baseline: 2.2412x; 2.2412x over previous
"""FFT depthwise conv == direct 7x7 circular depthwise conv, on 8 TRN2 cores.

out[b,i,j,c] = sum_{u,v} wf[c,u,v] * x[b,(i+u-3)%H,(j+v-3)%W,c],  wf = kernel[:, ::-1, ::-1]

Banded-matmul scheme: image ROWS live on SBUF partitions, so one matmul with a
7-diagonal banded stationary matrix covers all 7 row-taps (u) at once; the 7
column-taps (v) become 7 PSUM-accumulated matmuls whose rhs is the same tile
shifted by v columns.  49 taps in 7 matmuls instead of 49.

Sharding: channels (192/8 = 24 per core), so all 8 images stream through each
banded weight while it is stationary (weight-load overhead amortized 8x).

Per (channel, row-half): input tile [118 rows x (8 img x 230 cols)] bf16; for
v in 0..6: one matmul per image-pair q (PSUM bank q, [112 x 448] f32,
start=(v==0), stop=(v==6)).  lhsT[p, m] = wf[c, p-m, v] (7-diag band).
PSUM -> bf16 SBUF evac split across ScalarE/VectorE, one 401KB output DMA per
(c, half).  Host pre-builds the circularly-padded row-major tiles and banded
weights; host also reassembles the final (B,H,W,C) output.
"""

import os
import sys

for _p in ("/opt/trn_rl_repo", "/root/.axon_site/_ro/trn_rl_repo"):
    if os.path.isdir(_p) and _p not in sys.path:
        sys.path.insert(0, _p)

import numpy as np

import concourse.bacc as bacc
import concourse.bass as bass
import concourse.mybir as mybir
from concourse.bass_utils import run_bass_kernel_spmd
from concourse.tile import TileContext

F32 = mybir.dt.float32
BF16 = mybir.dt.bfloat16

B, H, W, C, K = 8, 224, 224, 192, 7
NCORES = 8
CPC = C // NCORES        # 24 channels per core
PAD = K // 2             # 3
HALFR = H // 2           # 112 output rows per half
PROWS = HALFR + 2 * PAD  # 118 input rows per half-tile (partitions)
NCOL = W + 2 * PAD       # 230 padded cols
NQ = 4                   # image pairs per (c, half): 4 x 2 = 8 images
QCOLS = 2 * W            # 448 psum cols per pair
PREFETCH = 2


def build_nc():
    nc = bacc.Bacc()
    x_d = nc.declare_dram_parameter("x", [CPC, 2, PROWS, B, NCOL], BF16, isOutput=False)
    w_d = nc.declare_dram_parameter("w", [PROWS, CPC, K, HALFR], BF16, isOutput=False)
    out_d = nc.declare_dram_parameter("out", [CPC, 2, HALFR, NQ, QCOLS], BF16, isOutput=True)

    mult = mybir.AluOpType.mult

    with TileContext(nc) as tc:
        with (
            tc.tile_pool(name="wp", bufs=1) as wpool,
            tc.tile_pool(name="xp", bufs=PREFETCH + 2) as xpool,
            tc.tile_pool(name="op", bufs=3) as opool,
            tc.tile_pool(name="pp", bufs=8, space="PSUM") as ppool,
        ):
            wsb = wpool.tile([PROWS, CPC, K, HALFR], BF16)
            # c=0 weights first (tiny) so compute can start ASAP; rest follows
            # on the same (scalar) queue, off the input-DMA (sync) queue.
            nc.scalar.dma_start(out=wsb[:, 0:1], in_=w_d[:, 0:1])
            nc.scalar.dma_start(out=wsb[:, 1:CPC], in_=w_d[:, 1:CPC])

            units = [(c, h) for c in range(CPC) for h in range(2)]
            pending = {}

            def issue_x(i):
                c, h = units[i]
                xt = xpool.tile([PROWS, B, NCOL], BF16, name=f"xt{c}_{h}", tag="xt")
                nc.sync.dma_start(out=xt[:], in_=x_d[c, h])
                pending[i] = xt

            for j in range(PREFETCH):
                issue_x(j)

            for i, (c, h) in enumerate(units):
                if i + PREFETCH < len(units):
                    issue_x(i + PREFETCH)
                xt = pending.pop(i)
                pss = []
                for q in range(NQ):
                    ps = ppool.tile([HALFR, QCOLS], F32, name=f"ps{c}_{h}_{q}", tag="ps")
                    pss.append(ps)
                for v in range(K):
                    wap = wsb[:, c, v, :]
                    for q in range(NQ):
                        nc.tensor.matmul(
                            pss[q][:],
                            wap,
                            xt[:, 2 * q : 2 * q + 2, v : v + W],
                            start=(v == 0),
                            stop=(v == K - 1),
                        )
                ot = opool.tile([HALFR, NQ, QCOLS], BF16, name=f"ot{c}_{h}", tag="ot")
                for q in range(NQ):
                    if q % 2 == 0:
                        nc.scalar.copy(out=ot[:, q, :], in_=pss[q][:])
                    else:
                        nc.vector.tensor_scalar(ot[:, q, :], pss[q][:], 1.0, None, mult)
                eng = nc.scalar if i % 2 == 0 else nc.sync
                eng.dma_start(out=out_d[c, h], in_=ot[:])
    return nc


def _host_pack_x(x):
    """x (B,H,W,C) f32 -> per-core [CPC, 2, PROWS, B, NCOL] bf16."""
    import ml_dtypes

    xt = np.transpose(x, (3, 1, 2, 0))  # (C, H, W, B)
    xt = np.concatenate([xt[:, :, -PAD:, :], xt, xt[:, :, :PAD, :]], axis=2)  # (C,H,230,B)
    halves = []
    for h in range(2):
        rows = (np.arange(PROWS) + h * HALFR - PAD) % H
        th = xt[:, rows]                       # (C, 118, 230, B)
        halves.append(np.transpose(th, (0, 1, 3, 2)))  # (C, 118, B, 230)
    xp = np.stack(halves, axis=1).astype(ml_dtypes.bfloat16)  # (C, 2, 118, B, 230)
    return [np.ascontiguousarray(xp[k * CPC : (k + 1) * CPC]) for k in range(NCORES)]


def _host_pack_w(kernel):
    """kernel (C,K,K) -> per-core banded lhsT [PROWS, CPC, K, HALFR] bf16.

    lhsT[p, cl, v, m] = wf[c0+cl, p-m, v] for 0 <= p-m < 7, else 0.
    """
    import ml_dtypes

    wf = np.ascontiguousarray(kernel[:, ::-1, ::-1]).astype(np.float32)  # (C, K, K)
    blobs = []
    m_idx = np.arange(HALFR)
    for k in range(NCORES):
        warr = np.zeros((PROWS, CPC, K, HALFR), dtype=np.float32)
        wc = wf[k * CPC : (k + 1) * CPC]  # (24, 7, 7)
        for u in range(K):
            # warr[m+u, :, v, m] = wc[:, u, v]
            warr[m_idx + u, :, :, m_idx] = wc[:, u, :]
        blobs.append(warr.astype(ml_dtypes.bfloat16))
    return blobs


_NC_CACHE = {}


def _get_nc():
    if "nc" not in _NC_CACHE:
        nc = build_nc()
        nc.finalize()
        _NC_CACHE["nc"] = nc
    return _NC_CACHE["nc"]


def run(x, kernel, trace=False, **kw):
    assert x.shape == (B, H, W, C) and kernel.shape == (C, K, K)
    nc = _get_nc()
    xs = _host_pack_x(np.asarray(x).astype(np.float32))
    ws = _host_pack_w(np.asarray(kernel))
    in_maps = [{"x": xs[k], "w": ws[k]} for k in range(NCORES)]
    res = run_bass_kernel_spmd(nc, in_maps, list(range(NCORES)), trace=trace, **kw)
    # out blob [CPC, 2, HALFR, NQ, 448] -> (B, H, W, CPC) per core
    parts = []
    for k in range(NCORES):
        o = np.asarray(res.results[k]["out"]).astype(np.float32)
        o = o.reshape(CPC, 2, HALFR, B, W)          # (c, h, m, img, j)
        o = np.transpose(o, (3, 1, 2, 4, 0))        # (img, h, m, j, c)
        parts.append(o.reshape(B, H, W, CPC))
    out = np.concatenate(parts, axis=3)
    return np.ascontiguousarray(out), res


def kernel(x, kernel):
    out, _ = run(np.asarray(x), np.asarray(kernel))
    return out


# revision 9
# speedup vs baseline: 3.7339x; 1.6661x over previous
"""FFT depthwise conv == direct 7x7 circular depthwise conv, on 8 TRN2 cores.

out[b,i,j,c] = sum_{u,v} wf[c,u,v] * x[b,(i+u-3)%H,(j+v-3)%W,c],  wf = kernel[:, ::-1, ::-1]

Banded-matmul scheme: image ROWS live on SBUF partitions, so one matmul with a
7-diagonal banded stationary matrix covers all 7 row-taps (u) at once; the 7
column-taps (v) become 7 PSUM-accumulated matmuls whose rhs is the same tile
shifted by v columns.  49 taps in 7 matmuls instead of 49.

Sharding: channels (192/8 = 24 per core), so all 8 images stream through each
banded weight while it is stationary (weight-load overhead amortized 8x).

Per (channel, row-half): input tile [118 rows x (8 img x 230 cols)] bf16; for
v in 0..6: one matmul per image-pair q (PSUM bank q, [112 x 448] f32,
start=(v==0), stop=(v==6)).  lhsT[p, m] = wf[c, p-m, v] (7-diag band).
PSUM -> bf16 SBUF evac split across ScalarE/VectorE, one 401KB output DMA per
(c, half).  Host pre-builds the circularly-padded row-major tiles and banded
weights; host also reassembles the final (B,H,W,C) output.
"""

import os
import sys

for _p in ("/opt/trn_rl_repo", "/root/.axon_site/_ro/trn_rl_repo"):
    if os.path.isdir(_p) and _p not in sys.path:
        sys.path.insert(0, _p)

import numpy as np

import concourse.bacc as bacc
import concourse.bass as bass
import concourse.mybir as mybir
from concourse.bass_utils import run_bass_kernel_spmd
from concourse.tile import TileContext

F32 = mybir.dt.float32
BF16 = mybir.dt.bfloat16

B, H, W, C, K = 8, 224, 224, 192, 7
NCORES = 8
CPC = C // NCORES        # 24 channels per core
PAD = K // 2             # 3
HALFR = H // 2           # 112 output rows per half
PROWS = HALFR + 2 * PAD  # 118 input rows actually used per half-tile
TROWS = 128              # tile partition rows (16-multiple so the DMA
                         # descriptor balancer sprays all 16 SDMA engines;
                         # 118 partitions degenerate to a 2-engine split)
NCOL = W + 2 * PAD       # 230 padded cols
NQ = 4                   # image pairs per (c, half): 4 x 2 = 8 images
QCOLS = 2 * W            # 448 psum cols per pair
PREFETCH = 2


def build_nc():
    nc = bacc.Bacc()
    x_d = nc.declare_dram_parameter("x", [CPC, 2, TROWS, B, NCOL], BF16, isOutput=False)
    w_d = nc.declare_dram_parameter("w", [TROWS, CPC, K, HALFR], BF16, isOutput=False)
    out_d = nc.declare_dram_parameter("out", [CPC, 2, HALFR, NQ, QCOLS], BF16, isOutput=True)

    mult = mybir.AluOpType.mult

    with TileContext(nc) as tc:
        with (
            tc.tile_pool(name="wp", bufs=1) as wpool,
            tc.tile_pool(name="xp", bufs=PREFETCH + 2) as xpool,
            tc.tile_pool(name="op", bufs=3) as opool,
            tc.tile_pool(name="pp", bufs=8, space="PSUM") as ppool,
        ):
            wsb = wpool.tile([TROWS, CPC, K, HALFR], BF16)
            # c=0 weights first (tiny) so compute can start ASAP; rest follows
            # on the same (scalar) queue, off the input-DMA (sync) queue.
            nc.scalar.dma_start(out=wsb[:, 0:1], in_=w_d[:, 0:1])
            nc.scalar.dma_start(out=wsb[:, 1:CPC], in_=w_d[:, 1:CPC])

            units = [(c, h) for c in range(CPC) for h in range(2)]
            pending = {}

            def issue_x(i):
                c, h = units[i]
                xt = xpool.tile([TROWS, B, NCOL], BF16, name=f"xt{c}_{h}", tag="xt")
                nc.sync.dma_start(out=xt[:], in_=x_d[c, h])
                pending[i] = xt

            for j in range(PREFETCH):
                issue_x(j)

            for i, (c, h) in enumerate(units):
                if i + PREFETCH < len(units):
                    issue_x(i + PREFETCH)
                xt = pending.pop(i)
                pss = []
                for q in range(NQ):
                    ps = ppool.tile([HALFR, QCOLS], F32, name=f"ps{c}_{h}_{q}", tag="ps")
                    pss.append(ps)
                for v in range(K):
                    wap = wsb[0:PROWS, c, v, :]
                    for q in range(NQ):
                        nc.tensor.matmul(
                            pss[q][:],
                            wap,
                            xt[0:PROWS, 2 * q : 2 * q + 2, v : v + W],
                            start=(v == 0),
                            stop=(v == K - 1),
                        )
                ot = opool.tile([HALFR, NQ, QCOLS], BF16, name=f"ot{c}_{h}", tag="ot")
                for q in range(NQ):
                    if q % 2 == 0:
                        nc.scalar.copy(out=ot[:, q, :], in_=pss[q][:])
                    else:
                        nc.vector.tensor_scalar(ot[:, q, :], pss[q][:], 1.0, None, mult)
                eng = nc.scalar if i % 2 == 0 else nc.sync
                eng.dma_start(out=out_d[c, h], in_=ot[:])
    return nc


def _host_pack_x(x):
    """x (B,H,W,C) f32 -> per-core [CPC, 2, TROWS, B, NCOL] bf16."""
    import ml_dtypes

    xt = np.transpose(x, (3, 1, 2, 0))  # (C, H, W, B)
    xt = np.concatenate([xt[:, :, -PAD:, :], xt, xt[:, :, :PAD, :]], axis=2)  # (C,H,230,B)
    halves = []
    for h in range(2):
        rows = (np.arange(TROWS) + h * HALFR - PAD) % H
        th = xt[:, rows]                       # (C, 128, 230, B)
        halves.append(np.transpose(th, (0, 1, 3, 2)))  # (C, 128, B, 230)
    xp = np.stack(halves, axis=1).astype(ml_dtypes.bfloat16)  # (C, 2, 128, B, 230)
    return [np.ascontiguousarray(xp[k * CPC : (k + 1) * CPC]) for k in range(NCORES)]


def _host_pack_w(kernel):
    """kernel (C,K,K) -> per-core banded lhsT [PROWS, CPC, K, HALFR] bf16.

    lhsT[p, cl, v, m] = wf[c0+cl, p-m, v] for 0 <= p-m < 7, else 0.
    """
    import ml_dtypes

    wf = np.ascontiguousarray(kernel[:, ::-1, ::-1]).astype(np.float32)  # (C, K, K)
    blobs = []
    m_idx = np.arange(HALFR)
    for k in range(NCORES):
        warr = np.zeros((TROWS, CPC, K, HALFR), dtype=np.float32)
        wc = wf[k * CPC : (k + 1) * CPC]  # (24, 7, 7)
        for u in range(K):
            # warr[m+u, :, v, m] = wc[:, u, v]
            warr[m_idx + u, :, :, m_idx] = wc[:, u, :]
        blobs.append(warr.astype(ml_dtypes.bfloat16))
    return blobs


_NC_CACHE = {}


def _get_nc():
    if "nc" not in _NC_CACHE:
        nc = build_nc()
        nc.finalize()
        _NC_CACHE["nc"] = nc
    return _NC_CACHE["nc"]


def run(x, kernel, trace=False, **kw):
    assert x.shape == (B, H, W, C) and kernel.shape == (C, K, K)
    nc = _get_nc()
    xs = _host_pack_x(np.asarray(x).astype(np.float32))
    ws = _host_pack_w(np.asarray(kernel))
    in_maps = [{"x": xs[k], "w": ws[k]} for k in range(NCORES)]
    res = run_bass_kernel_spmd(nc, in_maps, list(range(NCORES)), trace=trace, **kw)
    # out blob [CPC, 2, HALFR, NQ, 448] -> (B, H, W, CPC) per core
    parts = []
    for k in range(NCORES):
        o = np.asarray(res.results[k]["out"]).astype(np.float32)
        o = o.reshape(CPC, 2, HALFR, B, W)          # (c, h, m, img, j)
        o = np.transpose(o, (3, 1, 2, 4, 0))        # (img, h, m, j, c)
        parts.append(o.reshape(B, H, W, CPC))
    out = np.concatenate(parts, axis=3)
    return np.ascontiguousarray(out), res


def kernel(x, kernel):
    out, _ = run(np.asarray(x), np.asarray(kernel))
    return out


# revision 14
# speedup vs baseline: 4.5403x; 1.2160x over previous
"""FFT depthwise conv == direct 7x7 circular depthwise conv, on 8 TRN2 cores.

out[b,i,j,c] = sum_{u,v} wf[c,u,v] * x[b,(i+u-3)%H,(j+v-3)%W,c],  wf = kernel[:, ::-1, ::-1]

Banded-matmul scheme: image ROWS live on SBUF partitions, so one matmul with a
7-diagonal banded stationary matrix covers all 7 row-taps (u) at once; the 7
column-taps (v) become 7 PSUM-accumulated matmuls whose rhs is the same tile
shifted by v columns.  49 taps in 7 matmuls instead of 49.

Sharding: channels (192/8 = 24 per core), so all 8 images stream through each
banded weight while it is stationary (weight-load overhead amortized 8x).

Per (channel, row-half): input tile [118 rows x (8 img x 230 cols)] bf16; for
v in 0..6: one matmul per image-pair q (PSUM bank q, [112 x 448] f32,
start=(v==0), stop=(v==6)).  lhsT[p, m] = wf[c, p-m, v] (7-diag band).
PSUM -> bf16 SBUF evac split across ScalarE/VectorE, one 401KB output DMA per
(c, half).  Host pre-builds the circularly-padded row-major tiles and banded
weights; host also reassembles the final (B,H,W,C) output.
"""

import os
import sys

for _p in ("/opt/trn_rl_repo", "/root/.axon_site/_ro/trn_rl_repo"):
    if os.path.isdir(_p) and _p not in sys.path:
        sys.path.insert(0, _p)

import numpy as np

import concourse.bacc as bacc
import concourse.bass as bass
import concourse.mybir as mybir
from concourse.bass_utils import run_bass_kernel_spmd
from concourse.tile import TileContext

F32 = mybir.dt.float32
BF16 = mybir.dt.bfloat16

B, H, W, C, K = 8, 224, 224, 192, 7
NCORES = 8
CPC = C // NCORES        # 24 channels per core
PAD = K // 2             # 3
HALFR = H // 2           # 112 output rows per half
PROWS = HALFR + 2 * PAD  # 118 input rows actually used per half-tile
TROWS = 128              # tile partition rows (16-multiple so the DMA
                         # descriptor balancer sprays all 16 SDMA engines;
                         # 118 partitions degenerate to a 2-engine split)
WCOLS = 128              # banded lhsT padded to 128x128 so walrus enables
                         # FWL (fast weight load, 2 bf16/cycle) -- the
                         # per-matmul LDWEIGHTS then hides under the stream
NCOL = W + 2 * PAD       # 230 padded cols
NQ = 4                   # image pairs per (c, half): 4 x 2 = 8 images
QCOLS = 2 * W            # 448 psum cols per pair
PREFETCH = 2


def build_nc():
    nc = bacc.Bacc()
    x_d = nc.declare_dram_parameter("x", [CPC, 2, TROWS, B, NCOL], BF16, isOutput=False)
    w_d = nc.declare_dram_parameter("w", [TROWS, CPC, K, WCOLS], BF16, isOutput=False)
    out_d = nc.declare_dram_parameter("out", [CPC, 2, HALFR, NQ, QCOLS], BF16, isOutput=True)

    mult = mybir.AluOpType.mult

    with TileContext(nc) as tc:
        with (
            tc.tile_pool(name="wp", bufs=1) as wpool,
            tc.tile_pool(name="xp", bufs=PREFETCH + 2) as xpool,
            tc.tile_pool(name="op", bufs=3) as opool,
            tc.tile_pool(name="pp", bufs=8, space="PSUM") as ppool,
        ):
            wsb = wpool.tile([TROWS, CPC, K, WCOLS], BF16)
            # per-channel weight DMAs (c=0 first) so channel c's weights are
            # always resident well before its matmuls; one big blob DMA makes
            # c=1 stall ~5us at startup (and re-throttles HAM).  Scalar queue
            # keeps these off the input-DMA (sync) queue.
            for c in range(CPC):
                nc.scalar.dma_start(out=wsb[:, c : c + 1], in_=w_d[:, c : c + 1])

            units = [(c, h) for c in range(CPC) for h in range(2)]
            pending = {}

            def issue_x(i):
                c, h = units[i]
                xt = xpool.tile([TROWS, B, NCOL], BF16, name=f"xt{c}_{h}", tag="xt")
                nc.sync.dma_start(out=xt[:], in_=x_d[c, h])
                pending[i] = xt

            for j in range(PREFETCH):
                issue_x(j)

            for i, (c, h) in enumerate(units):
                if i + PREFETCH < len(units):
                    issue_x(i + PREFETCH)
                xt = pending.pop(i)
                pss = []
                for q in range(NQ):
                    ps = ppool.tile([TROWS, QCOLS], F32, name=f"ps{c}_{h}_{q}", tag="ps")
                    pss.append(ps)
                for v in range(K):
                    wap = wsb[:, c, v, :]
                    for q in range(NQ):
                        nc.tensor.matmul(
                            pss[q][:],
                            wap,
                            xt[:, 2 * q : 2 * q + 2, v : v + W],
                            start=(v == 0),
                            stop=(v == K - 1),
                        )
                ot = opool.tile([HALFR, NQ, QCOLS], BF16, name=f"ot{c}_{h}", tag="ot")
                for q in range(NQ):
                    if q % 2 == 0:
                        nc.scalar.copy(out=ot[:, q, :], in_=pss[q][0:HALFR, :])
                    else:
                        nc.vector.tensor_scalar(ot[:, q, :], pss[q][0:HALFR, :], 1.0, None, mult)
                eng = nc.scalar if i % 2 == 0 else nc.sync
                eng.dma_start(out=out_d[c, h], in_=ot[:])
    return nc


def _host_pack_x(x):
    """x (B,H,W,C) f32 -> per-core [CPC, 2, TROWS, B, NCOL] bf16."""
    import ml_dtypes

    xt = np.transpose(x, (3, 1, 2, 0))  # (C, H, W, B)
    xt = np.concatenate([xt[:, :, -PAD:, :], xt, xt[:, :, :PAD, :]], axis=2)  # (C,H,230,B)
    halves = []
    for h in range(2):
        rows = (np.arange(TROWS) + h * HALFR - PAD) % H
        th = xt[:, rows]                       # (C, 128, 230, B)
        halves.append(np.transpose(th, (0, 1, 3, 2)))  # (C, 128, B, 230)
    xp = np.stack(halves, axis=1).astype(ml_dtypes.bfloat16)  # (C, 2, 128, B, 230)
    return [np.ascontiguousarray(xp[k * CPC : (k + 1) * CPC]) for k in range(NCORES)]


def _host_pack_w(kernel):
    """kernel (C,K,K) -> per-core banded lhsT [PROWS, CPC, K, HALFR] bf16.

    lhsT[p, cl, v, m] = wf[c0+cl, p-m, v] for 0 <= p-m < 7, else 0.
    """
    import ml_dtypes

    wf = np.ascontiguousarray(kernel[:, ::-1, ::-1]).astype(np.float32)  # (C, K, K)
    blobs = []
    m_idx = np.arange(HALFR)
    for k in range(NCORES):
        warr = np.zeros((TROWS, CPC, K, WCOLS), dtype=np.float32)
        wc = wf[k * CPC : (k + 1) * CPC]  # (24, 7, 7)
        for u in range(K):
            # warr[m+u, :, v, m] = wc[:, u, v]
            warr[m_idx + u, :, :, m_idx] = wc[:, u, :]
        blobs.append(warr.astype(ml_dtypes.bfloat16))
    return blobs


_NC_CACHE = {}


def _get_nc():
    if "nc" not in _NC_CACHE:
        nc = build_nc()
        nc.finalize()
        _NC_CACHE["nc"] = nc
    return _NC_CACHE["nc"]


def run(x, kernel, trace=False, **kw):
    assert x.shape == (B, H, W, C) and kernel.shape == (C, K, K)
    nc = _get_nc()
    xs = _host_pack_x(np.asarray(x).astype(np.float32))
    ws = _host_pack_w(np.asarray(kernel))
    in_maps = [{"x": xs[k], "w": ws[k]} for k in range(NCORES)]
    res = run_bass_kernel_spmd(nc, in_maps, list(range(NCORES)), trace=trace, **kw)
    # out blob [CPC, 2, HALFR, NQ, 448] -> (B, H, W, CPC) per core
    parts = []
    for k in range(NCORES):
        o = np.asarray(res.results[k]["out"]).astype(np.float32)
        o = o.reshape(CPC, 2, HALFR, B, W)          # (c, h, m, img, j)
        o = np.transpose(o, (3, 1, 2, 4, 0))        # (img, h, m, j, c)
        parts.append(o.reshape(B, H, W, CPC))
    out = np.concatenate(parts, axis=3)
    return np.ascontiguousarray(out), res


def kernel(x, kernel):
    out, _ = run(np.asarray(x), np.asarray(kernel))
    return out
